# revision 3
# baseline (speedup 1.0000x reference)
"""Multi-head attention (softmax+1) for TRN2, 8 NeuronCores.

Sharding: data-parallel over batch B=2 x tensor-parallel over the 16 heads
(4 heads per core).  Each core computes its 4 heads' QKV projections,
attention, and a partial output projection; the host sums the 4 partials
per batch and adds the output bias.

v2: the attention phase is scalar-engine (exp) bound, so everything is
organized around keeping ACT busy from ~9us on with 1280-element
activation instructions:
  - scores land in a 5-bank PSUM ring [128, 2560] f32; exp reads 1280-elem
    chunks (amortizes the ~220-cycle ACT access latency better than 1024);
  - exp output goes to a 25-slot SBUF ring [128, 25*1024] f16; the V'@exp
    accumulation (AV) drains it lazily (up to ~25 ktiles behind), which
    lets the first quarters spend PE time on K/Q/V projection chunks;
  - K/Q/V projection chunks and out-projection halves are "aux units"
    interleaved between score steps through one shared PSUM bank;
  - PSUM: 5 (ring) + 2 (AV accum) + 1 (aux) = 8 banks.
Quarter order alternates head pairs, (0,0),(1,0),(0,1),(1,1),..., so the
out-projection (which needs both pairs at the same q) starts mid-run.
"""

import sys

if "/opt/trn_rl_repo" not in sys.path:
    sys.path.insert(0, "/opt/trn_rl_repo")

import numpy as np

import concourse.bass as bass
import concourse.mybir as mybir
import concourse.tile as tile
from concourse import bacc
from concourse.bass_utils import run_bass_kernel_spmd

F32 = mybir.dt.float32
F16 = mybir.dt.float16
EXP = mybir.ActivationFunctionType.Exp

B, S, DM = 2, 2048, 1024
H, HD = 16, 64
SCALE = HD ** -0.5
HLOC = 4              # heads per core
CD = HLOC * HD        # 256 local head dims
MC = DM // 128        # 8 contraction chunks for projections
NKT = S // 128        # 16 k tiles per quarter sweep
NQC = S // 512        # 4 input column chunks

QUARTERS = [(0, 0), (1, 0), (0, 1), (1, 1), (0, 2), (1, 2), (0, 3), (1, 3)]
NSTEPS = 8 * NKT                 # 128 ktile steps
NCHUNKS = 2 * NSTEPS             # 256 score chunks of [128, 512]
RING_F32 = 2560                  # 5 PSUM banks
ERING_SLOTS = 25                 # exp ring slots of 1024 f16 (50 chunks)
ERING_F16 = ERING_SLOTS * 1024
ACT_N = 1280

_CACHE = {}
LAST_RESULT = None


def _build():
    nc = bacc.Bacc()
    dp = nc.declare_dram_parameter
    xq_d = dp("xq", [NQC, DM, 512], F16, isOutput=False)   # q-chunked query^T
    xk_d = dp("xk", [NQC, DM, 512], F16, isOutput=False)   # k-chunked key^T
    xv_d = dp("xv", [NQC, DM, 512], F16, isOutput=False)   # k-chunked value^T
    wq_d = dp("wq", [DM, CD], F16, isOutput=False)         # (SCALE*Wq_shard)^T
    wk_d = dp("wk", [DM, CD], F16, isOutput=False)
    wv_d = dp("wv", [DM, CD], F16, isOutput=False)         # Wv_shard^T
    wo_d = dp("wo", [CD, DM], F16, isOutput=False)
    bq_d = dp("bq", [128, 2], F32, isOutput=False)         # bias col per 128-pair
    bk_d = dp("bk", [128, 2], F32, isOutput=False)
    bv_d = dp("bv", [1, HLOC, HD], F16, isOutput=False)
    out_d = dp("out", [S, DM], F16, isOutput=True)         # partial (pre-bo)

    with tile.TileContext(nc) as tc:
        with tc.tile_pool(name="weights", bufs=1) as wpool, \
             tc.tile_pool(name="persist", bufs=1) as perst, \
             tc.tile_pool(name="xs", bufs=40) as xs, \
             tc.tile_pool(name="obuf", bufs=3) as obuf, \
             tc.tile_pool(name="npool", bufs=2) as npool:
            wq_sb = wpool.tile([128, MC, CD], F16)
            wk_sb = wpool.tile([128, MC, CD], F16)
            wv_sb = wpool.tile([128, MC, CD], F16)
            wo_sb = wpool.tile([128, 2, DM], F16)
            bq_sb = wpool.tile([128, 2], F32)
            bk_sb = wpool.tile([128, 2], F32)
            bv_sb = wpool.tile([1, HLOC, HD], F16)

            qt_sb = perst.tile([128, 2, S], F16)    # [d(2 heads), pair, q]
            kt_sb = perst.tile([128, 2, S], F16)
            v_sb = perst.tile([128, NKT, HLOC, HD + 1], F16)  # ones col at 64
            xv_sb = perst.tile([128, MC, S], F16)   # resident value^T chunks
            at_sb = perst.tile([128, 2, S], F16)    # normalized attn out^T
            ering = perst.tile([128, ERING_F16], F16)
            bvbc = perst.tile([128, HLOC, HD], F16)  # bv broadcast

            # ---------------- DMA helpers -----------------
            def dma_w(sb, dram, width):
                for m in range(MC):
                    nc.sync.dma_start(out=sb[:, m, :],
                                      in_=dram.ap()[m * 128:(m + 1) * 128, 0:width])

            xin_tiles = {}

            def stage_x(kind, c):
                if (kind, c) in xin_tiles:
                    return xin_tiles[(kind, c)]
                ts = [xs.tile([128, 512], F16, tag="xs", name=f"x{kind}{c}_{m}")
                      for m in range(MC)]
                dram = xk_d if kind == "k" else xq_d
                for m in range(MC):
                    nc.sync.dma_start(out=ts[m][:],
                                      in_=dram.ap()[c, m * 128:(m + 1) * 128, :])
                xin_tiles[(kind, c)] = ts
                return ts

            def dma_xv_chunk(c):
                for m in range(MC):
                    nc.sync.dma_start(out=xv_sb[:, m, c * 512:(c + 1) * 512],
                                      in_=xv_d.ap()[c, m * 128:(m + 1) * 128, :])

            # ---------------- startup: weights + first chunks ----------------
            dma_w(wk_sb, wk_d, CD)
            stage_x("k", 0)
            nc.sync.dma_start(out=bk_sb[:], in_=bk_d.ap())
            dma_w(wq_sb, wq_d, CD)
            stage_x("q", 0)
            nc.sync.dma_start(out=bq_sb[:], in_=bq_d.ap())

            # ---------------- aux unit bodies -----------------
            def kq_proj(aux, kind, p, c):
                w_sb, b_sb, dst = (wk_sb, bk_sb, kt_sb) if kind == "k" \
                    else (wq_sb, bq_sb, qt_sb)
                xt = xin_tiles[(kind, c)]
                ps = aux.tile([128, 512], F32, tag="aux", name="kqps")
                for m in range(MC):
                    nc.tensor.matmul(ps[:], w_sb[:, m, p * 128:(p + 1) * 128],
                                     xt[m][:], start=(m == 0), stop=(m == MC - 1))
                nc.vector.tensor_scalar_add(dst[:, p, c * 512:(c + 1) * 512],
                                            ps[:], b_sb[:, p:p + 1])

            def v_proj_pair(aux, j):
                vp = aux.tile([128, 2, HLOC, HD], F32, tag="aux", name="vps")
                for t in range(2):
                    kt = 2 * j + t
                    for m in range(MC):
                        nc.tensor.matmul(
                            vp[:, t],
                            xv_sb[:, m, kt * 128:(kt + 1) * 128],
                            wv_sb[:, m, :],
                            start=(m == 0), stop=(m == MC - 1),
                        )
                for t in range(2):
                    kt = 2 * j + t
                    nc.vector.tensor_add(v_sb[:, kt, :, 0:HD], vp[:, t], bvbc[:])

            ob_tiles = {}

            def out_proj_half(aux, t, n, act_copy=False):
                ob = ob_tiles.get(t)
                if ob is None:
                    ob = obuf.tile([128, DM], F16, tag="ob", name="ob")
                    ob_tiles[t] = ob
                op = aux.tile([128, 512], F32, tag="aux", name="ops")
                for cc in range(2):
                    nc.tensor.matmul(
                        op[:],
                        at_sb[:, cc, t * 128:(t + 1) * 128],
                        wo_sb[:, cc, n * 512:(n + 1) * 512],
                        start=(cc == 0), stop=(cc == 1),
                    )
                if act_copy:
                    nc.scalar.copy(ob[:, n * 512:(n + 1) * 512], op[:])
                else:
                    nc.vector.tensor_copy(ob[:, n * 512:(n + 1) * 512], op[:])
                if n == 1:
                    nc.sync.dma_start(out=out_d.ap()[t * 128:(t + 1) * 128, :],
                                      in_=ob[:])
                    del ob_tiles[t]

            # ---------------- pre-loop: first K and Q chunks -----------------
            with tc.tile_pool(name="boot", bufs=2, space="PSUM") as boot:
                bk0 = boot.tile([128, 512], F32, tag="b", name="bk0")
                xkt = xin_tiles[("k", 0)]
                for m in range(MC):
                    nc.tensor.matmul(bk0[:], wk_sb[:, m, 0:128], xkt[m][:],
                                     start=(m == 0), stop=(m == MC - 1))
                nc.vector.tensor_scalar_add(kt_sb[:, 0, 0:512], bk0[:],
                                            bk_sb[:, 0:1])
                bq0 = boot.tile([128, 512], F32, tag="b", name="bq0")
                xqt = xin_tiles[("q", 0)]
                for m in range(MC):
                    nc.tensor.matmul(bq0[:], wq_sb[:, m, 0:128], xqt[m][:],
                                     start=(m == 0), stop=(m == MC - 1))
                nc.vector.tensor_scalar_add(qt_sb[:, 0, 0:512], bq0[:],
                                            bq_sb[:, 0:1])

            # bulk DMAs in priority order
            stage_x("k", 1)
            nc.sync.dma_start(out=bv_sb[:], in_=bv_d.ap())
            dma_w(wv_sb, wv_d, CD)
            stage_x("k", 2)
            stage_x("k", 3)
            nc.gpsimd.partition_broadcast(bvbc[:], bv_sb[:])
            nc.vector.memset(v_sb[:, :, :, HD], 1.0)
            dma_xv_chunk(0)
            dma_xv_chunk(1)
            stage_x("q", 1)
            dma_xv_chunk(2)
            stage_x("q", 2)
            dma_xv_chunk(3)
            stage_x("q", 3)
            for cc in range(2):
                nc.sync.dma_start(out=wo_sb[:, cc, :],
                                  in_=wo_d.ap()[cc * 128:(cc + 1) * 128, :])

            # ---------------- main loop -----------------
            with tc.tile_pool(name="ringp", bufs=1, space="PSUM") as ringp, \
                 tc.tile_pool(name="utp", bufs=2, space="PSUM") as utp, \
                 tc.tile_pool(name="auxp", bufs=1, space="PSUM") as auxp:
                ring = ringp.tile([128, RING_F32], F32)

                aux_sched = [[] for _ in range(8)]

                def A(q, f, *a):
                    aux_sched[q].append((f, a))

                A(0, kq_proj, "k", 0, 1)
                A(0, kq_proj, "k", 0, 2)
                A(0, kq_proj, "k", 1, 0)
                A(0, kq_proj, "k", 0, 3)
                A(0, kq_proj, "k", 1, 1)
                A(0, kq_proj, "q", 1, 0)
                A(1, kq_proj, "k", 1, 2)
                A(1, kq_proj, "k", 1, 3)
                A(1, v_proj_pair, 0)
                A(1, v_proj_pair, 1)
                A(1, kq_proj, "q", 0, 1)
                A(1, v_proj_pair, 2)
                A(1, v_proj_pair, 3)
                A(2, v_proj_pair, 4)
                A(2, v_proj_pair, 5)
                A(2, v_proj_pair, 6)
                A(2, v_proj_pair, 7)
                A(2, kq_proj, "q", 1, 1)
                A(3, kq_proj, "q", 0, 2)
                A(4, kq_proj, "q", 1, 2)
                A(5, kq_proj, "q", 0, 3)
                A(6, kq_proj, "q", 1, 3)

                av_queue = [(q, i) for q in range(8) for i in range(NKT)]
                av_pos = 0
                cur_uts = None
                vdone = 0
                norm_done = set()
                oproj_pending = [(t, n) for t in range(16) for n in range(2)]
                oproj_pos = 0

                def normalize(uts, p, qq):
                    for hh in range(2):
                        po = 64 * hh
                        den1 = npool.tile([1, 512], F32, tag="den", name="den")
                        nc.vector.tensor_scalar_add(den1[:], uts[hh][64:65, :], 1.0)
                        r = npool.tile([1, 512], F32, tag="r", name="r")
                        nc.vector.reciprocal_approx_fast(r[:], den1[:])
                        rb = npool.tile([64, 512], F32, tag="rb", name="rb")
                        nc.gpsimd.partition_broadcast(rb[:], r[:])
                        nc.vector.tensor_mul(
                            at_sb[po:po + 64, p, qq * 512:qq * 512 + 512],
                            uts[hh][0:64, :], rb[:])
                    norm_done.add((p, qq))

                acts_done_steps = 0
                next_act = 0

                def emit_av(limit):
                    nonlocal av_pos, cur_uts
                    n = 0
                    while n < limit and av_pos < len(av_queue):
                        qv, iv = av_queue[av_pos]
                        if 16 * qv + iv >= acts_done_steps:
                            return
                        if iv // 2 >= vdone:
                            return
                        pv, qqv = QUARTERS[qv]
                        if iv == 0:
                            cur_uts = (
                                utp.tile([65, 512], F32, tag="ut", name="ut0"),
                                utp.tile([65, 512], F32, tag="ut", name="ut1"),
                            )
                        c = 2 * (16 * qv + iv)
                        for hh in range(2):
                            eoff = ((c + hh) % (2 * ERING_SLOTS)) * 512
                            h = 2 * pv + hh
                            nc.tensor.matmul(
                                cur_uts[hh][:],
                                v_sb[:, iv, h, :],
                                ering[:, eoff:eoff + 512],
                                start=(iv == 0), stop=(iv == NKT - 1),
                            )
                        if iv == NKT - 1:
                            normalize(cur_uts, pv, qqv)
                        av_pos += 1
                        n += 1

                def emit_oproj(limit, act_copy=False):
                    nonlocal oproj_pos
                    n = 0
                    while n < limit and oproj_pos < len(oproj_pending):
                        t, nn_ = oproj_pending[oproj_pos]
                        qq = t // 4
                        if (0, qq) not in norm_done or (1, qq) not in norm_done:
                            return
                        out_proj_half(auxp, t, nn_, act_copy)
                        oproj_pos += 1
                        n += 1

                for g in range(NSTEPS):
                    q, i = divmod(g, NKT)
                    p, qq = QUARTERS[q]
                    q0 = qq * 512
                    for hh in range(2):
                        c = 2 * g + hh
                        roff = (c % 5) * 512
                        nc.tensor.matmul(
                            ring[:, roff:roff + 512],
                            kt_sb[64 * hh:64 * hh + 64, p, i * 128:(i + 1) * 128],
                            qt_sb[64 * hh:64 * hh + 64, p, q0:q0 + 512],
                            start=True, stop=True,
                        )
                    while (ACT_N * (next_act + 1) - 1) // 512 <= 2 * g + 1:
                        j = next_act
                        nc.scalar.activation(
                            out=ering[:, (j * ACT_N) % ERING_F16:
                                      (j * ACT_N) % ERING_F16 + ACT_N],
                            in_=ring[:, (j * ACT_N) % RING_F32:
                                     (j * ACT_N) % RING_F32 + ACT_N],
                            func=EXP)
                        next_act += 1
                        acts_done_steps = (ACT_N * next_act) // 1024
                    if aux_sched[q]:
                        f, a = aux_sched[q].pop(0)
                        f(auxp, *a)
                        if f is v_proj_pair:
                            vdone = max(vdone, a[0] + 1)
                    emit_av(2)
                    emit_oproj(1)

                # tail: final partial ACT, leftover AV + out-proj
                if ACT_N * next_act < NCHUNKS * 512:
                    j = next_act
                    rem = NCHUNKS * 512 - ACT_N * j
                    nc.scalar.activation(
                        out=ering[:, (j * ACT_N) % ERING_F16:
                                  (j * ACT_N) % ERING_F16 + rem],
                        in_=ring[:, (j * ACT_N) % RING_F32:
                                 (j * ACT_N) % RING_F32 + rem],
                        func=EXP)
                    next_act += 1
                    acts_done_steps = NSTEPS
                while av_pos < len(av_queue):
                    prev = av_pos
                    emit_av(4)
                    emit_oproj(2, act_copy=True)
                    if av_pos == prev:
                        raise RuntimeError("AV emission stuck")
                emit_oproj(100, act_copy=True)
                assert oproj_pos == len(oproj_pending)

    nc.finalize()
    return nc


def kernel(query, key, value, Wq, bq, Wk, bk, Wv, bv, Wo, bo):
    global LAST_RESULT
    if "nc" not in _CACHE:
        _CACHE["nc"] = _build()
    nc = _CACHE["nc"]

    query = np.asarray(query, np.float32)
    key = np.asarray(key, np.float32)
    value = np.asarray(value, np.float32)
    Wq = np.asarray(Wq, np.float32)
    Wk = np.asarray(Wk, np.float32)
    Wv = np.asarray(Wv, np.float32)
    Wo = np.asarray(Wo, np.float32)
    bq = np.asarray(bq, np.float32)
    bk = np.asarray(bk, np.float32)
    bv = np.asarray(bv, np.float32)
    bo = np.asarray(bo, np.float32)

    def chunk_xt(x_b):  # [S, DM] -> [NQC, DM, 512] f16 (x^T column chunks)
        xt = x_b.T.astype(np.float16)
        return np.ascontiguousarray(xt.reshape(DM, NQC, 512).transpose(1, 0, 2))

    xq_c = [chunk_xt(query[b]) for b in range(B)]
    xk_c = [chunk_xt(key[b]) for b in range(B)]
    xv_c = [chunk_xt(value[b]) for b in range(B)]

    in_maps = []
    for c in range(8):
        b, hg = c // 4, c % 4
        r0 = hg * CD
        wq_s = np.ascontiguousarray((Wq[r0:r0 + CD, :] * SCALE).T).astype(np.float16)
        wk_s = np.ascontiguousarray(Wk[r0:r0 + CD, :].T).astype(np.float16)
        wv_s = np.ascontiguousarray(Wv[r0:r0 + CD, :].T).astype(np.float16)
        wo_s = np.ascontiguousarray(Wo[:, r0:r0 + CD].T).astype(np.float16)
        bq_s = np.ascontiguousarray((bq[r0:r0 + CD] * SCALE).reshape(2, 128).T)
        bk_s = np.ascontiguousarray(bk[r0:r0 + CD].reshape(2, 128).T)
        bv_s = np.ascontiguousarray(bv[r0:r0 + CD].reshape(1, HLOC, HD)).astype(np.float16)
        in_maps.append({
            "xq": xq_c[b], "xk": xk_c[b], "xv": xv_c[b],
            "wq": wq_s, "wk": wk_s, "wv": wv_s, "wo": wo_s,
            "bq": bq_s, "bk": bk_s, "bv": bv_s,
        })

    res = run_bass_kernel_spmd(nc, in_maps, core_ids=list(range(8)))
    LAST_RESULT = res

    out = np.empty((B, S, DM), np.float32)
    for b in range(B):
        acc = np.zeros((S, DM), np.float64)
        for hg in range(4):
            acc += res.results[b * 4 + hg]["out"].astype(np.float64)
        out[b] = (acc + bo.astype(np.float64)).astype(np.float32)
    return out


# revision 8
# speedup vs baseline: 1.4625x; 1.4625x over previous
"""Multi-head attention (softmax+1) for TRN2, 8 NeuronCores.

Sharding: data-parallel over batch B=2 (4 cores per batch) x tensor-parallel
over the 16 heads (4 heads per core).  Each core computes its 4 heads'
QKV projections, attention, and a partial output projection; the host sums
the 4 partials per batch and adds the output bias.

Per-core kernel (S=2048, DM=1024, HD=64, Hloc=4):
  QT[d,q] / KT[d,k] head-transposed layouts from x^T inputs (PE matmuls),
  V'[k, 4*65] natural layout with a ones column per head (denominator trick),
  scores^T[k,q] -> exp on ACT (scale folded into Wq) -> U^T = V'^T @ expT
  (row 64 of each head's block = softmax denominator), normalization via
  1/(1+den) broadcast (GPSIMD partition_broadcast), partial out-projection.

All matmuls run in float16 (1 cycle/row on the PE).  Matmuls are emitted in
concurrent row-group pairs wherever possible (head-pair scores on partitions
0:64/64:128; projections as half-contraction pairs alternating row groups),
which hides LDWEIGHTS and doubles array occupancy.  The attention phase is
ACT(exp)-bound and software-pipelined one chunk ahead (scores/exp lead the
V-accumulation) so the scalar engine never starves across quarter
boundaries; V-projection / out-projection / dummy matmuls fill the PE to
keep the HAM clock-gate at 8/8.
"""

import sys

if "/opt/trn_rl_repo" not in sys.path:
    sys.path.insert(0, "/opt/trn_rl_repo")

import numpy as np

import concourse.bass as bass
import concourse.mybir as mybir
import concourse.tile as tile
from concourse import bacc
from concourse.bass_utils import run_bass_kernel_spmd

F32 = mybir.dt.float32
F16 = mybir.dt.float16
EXP = mybir.ActivationFunctionType.Exp

B, S, DM = 2, 2048, 1024
H, HD = 16, 64
SCALE = HD ** -0.5
HLOC = 4              # heads per core
CD = HLOC * HD        # 256 local head dims
VW = HD + 1           # 65: V columns + ones column per head
MC = DM // 128        # 8 contraction chunks for projections
KT16 = S // 128       # 16 sequence tiles
W260 = HLOC * VW      # 260

_CACHE = {}
LAST_RESULT = None


def _build():
    nc = bacc.Bacc()
    dp = nc.declare_dram_parameter
    xq_d = dp("xq", [DM, S], F16, isOutput=False)    # query[b]^T
    xk_d = dp("xk", [DM, S], F16, isOutput=False)
    xv_d = dp("xv", [DM, S], F16, isOutput=False)
    wq_d = dp("wq", [DM, CD], F16, isOutput=False)   # (SCALE * Wq_shard)^T
    wk_d = dp("wk", [DM, CD], F16, isOutput=False)   # Wk_shard^T
    wv_d = dp("wv", [DM, W260], F16, isOutput=False)  # Wv^T 260-layout, zeros in ones-cols
    wo_d = dp("wo", [CD, DM], F16, isOutput=False)   # Wo_shard^T
    bq_d = dp("bq", [128, 2], F32, isOutput=False)   # bias cols per 128-pair (SCALE-folded)
    bk_d = dp("bk", [128, 2], F32, isOutput=False)
    bv_d = dp("bv", [1, W260], F16, isOutput=False)  # [bv_h | 1.0] blocks
    on_d = dp("ones1", [1, 128], F16, isOutput=False)
    out_d = dp("out", [S, DM], F16, isOutput=True)   # partial (pre-bo) projection

    with tile.TileContext(nc) as tc:
        with tc.tile_pool(name="weights", bufs=1) as wpool, \
             tc.tile_pool(name="persist", bufs=1) as perst:
            wq_sb = wpool.tile([128, MC, CD], F16)
            wk_sb = wpool.tile([128, MC, CD], F16)
            wv_sb = wpool.tile([128, MC, W260], F16)
            wo_sb = wpool.tile([128, 2, DM], F16)
            bq_sb = wpool.tile([128, 2], F32)
            bk_sb = wpool.tile([128, 2], F32)
            bv_sb = wpool.tile([1, W260], F16)
            on_sb = wpool.tile([1, 128], F16)

            qt_sb = perst.tile([128, 2, S], F16)   # [d(2 heads), pair, q]
            kt_sb = perst.tile([128, 2, S], F16)
            v_sb = perst.tile([128, KT16, W260], F16)  # [k, ktile, 4*(V|1)]
            at_sb = perst.tile([128, 2, S], F16)   # normalized attn out^T
            xv_sb = perst.tile([128, MC, S], F16)  # resident value^T chunks

            # ------------- Phase 1: Q and K projections ----------------
            # Half-contraction matmul pairs on alternating row groups: the
            # second matmul's LDWEIGHTS overlaps the first's stream.
            with tc.tile_pool(name="xs", bufs=16) as xs, \
                 tc.tile_pool(name="pproj", bufs=8, space="PSUM") as pproj:
                nc.sync.dma_start(out=wq_sb[:, 0, :], in_=wq_d.ap()[0:128, :])
                for src_d, w_sb, b_sb, dst in (
                    (xq_d, wq_sb, bq_sb, qt_sb),
                    (xk_d, wk_sb, bk_sb, kt_sb),
                ):
                    first_proj = dst is qt_sb
                    pss = [pproj.tile([128, 512], F32, tag="ps", name=f"ps{k}")
                           for k in range(8)]
                    xts = []
                    for m in range(MC):
                        xt = xs.tile([128, S], F16, tag="xs", name=f"xt{m}")
                        nc.sync.dma_start(out=xt[:], in_=src_d.ap()[m * 128:(m + 1) * 128, :])
                        xts.append(xt)
                        if first_proj and m + 1 < MC:
                            nc.sync.dma_start(out=wq_sb[:, m + 1, :],
                                              in_=wq_d.ap()[(m + 1) * 128:(m + 2) * 128, :])
                    for m in range(MC):
                        xt = xts[m]
                        if first_proj and m == 0:
                            nc.sync.dma_start(out=bq_sb[:], in_=bq_d.ap())
                        st, sp = (m == 0), (m == MC - 1)
                        for p in range(2):
                            for j in range(4):
                                nc.tensor.matmul(
                                    pss[p * 4 + j][:],
                                    w_sb[:, m, p * 128:(p + 1) * 128],
                                    xt[:, j * 512:(j + 1) * 512],
                                    start=st, stop=sp,
                                )
                        if first_proj and m < 2:
                            for mm in range(m * 4, m * 4 + 4):
                                nc.sync.dma_start(out=wk_sb[:, mm, :],
                                                  in_=wk_d.ap()[mm * 128:(mm + 1) * 128, :])
                            if m == 0:
                                nc.sync.dma_start(out=bk_sb[:], in_=bk_d.ap())
                    for p in range(2):
                        for j in range(4):
                            nc.vector.tensor_scalar_add(
                                dst[:, p, j * 512:(j + 1) * 512],
                                pss[p * 4 + j][:], b_sb[:, p:p + 1],
                            )
                # stage V weights/input + wo for the attention phase
                for m in range(MC):
                    nc.sync.dma_start(out=wv_sb[:, m, :], in_=wv_d.ap()[m * 128:(m + 1) * 128, :])
                nc.sync.dma_start(out=bv_sb[:], in_=bv_d.ap())
                nc.sync.dma_start(out=on_sb[:], in_=on_d.ap())
                for m in range(MC):
                    nc.sync.dma_start(out=xv_sb[:, m, :], in_=xv_d.ap()[m * 128:(m + 1) * 128, :])
                for cc in range(2):
                    nc.sync.dma_start(out=wo_sb[:, cc, :], in_=wo_d.ap()[cc * 128:(cc + 1) * 128, :])

            # ------------- Phase 2: attention, software-pipelined -----------
            with tc.tile_pool(name="psc", bufs=2, space="PSUM") as psc, \
                 tc.tile_pool(name="put", bufs=2, space="PSUM") as put, \
                 tc.tile_pool(name="expp", bufs=4) as expp, \
                 tc.tile_pool(name="obuf", bufs=3) as obuf, \
                 tc.tile_pool(name="npool", bufs=3) as npool:

                pout = None
                pv_ctx = tc.tile_pool(name="pv", bufs=2, space="PSUM")
                pv = pv_ctx.__enter__()

                def vproj_pair(k0):
                    """V projection for k-tiles k0 and k0+1."""
                    for kt in (k0, k0 + 1):
                        vps = pv.tile([128, W260], F32, tag="vps", name="vps")
                        nc.tensor.matmul(vps[:], on_sb[:], bv_sb[:], start=True, stop=False)
                        for m in range(MC):
                            nc.tensor.matmul(
                                vps[:],
                                xv_sb[:, m, kt * 128:(kt + 1) * 128],
                                wv_sb[:, m, :],
                                start=False, stop=(m == MC - 1),
                            )
                        nc.vector.tensor_copy(v_sb[:, kt, :], vps[:])

                def outproj_t(t, act_copy=False):
                    ob = obuf.tile([128, DM], F16, tag="ob", name="ob")
                    ops = [pout.tile([128, 512], F32, tag="op", name=f"op{n}")
                           for n in range(2)]
                    for cc in range(2):
                        for n in range(2):
                            nc.tensor.matmul(
                                ops[n][:],
                                at_sb[:, cc, t * 128:(t + 1) * 128],
                                wo_sb[:, cc, n * 512:(n + 1) * 512],
                                start=(cc == 0), stop=(cc == 1),
                            )
                    nc.vector.tensor_copy(ob[:, 0:512], ops[0][:])
                    if act_copy:
                        nc.scalar.copy(ob[:, 512:1024], ops[1][:])
                    else:
                        nc.vector.tensor_copy(ob[:, 512:1024], ops[1][:])
                    nc.sync.dma_start(
                        out=out_d.ap()[t * 128:(t + 1) * 128, :], in_=ob[:],
                    )

                def dummy_mm():
                    wps = pout.tile([128, 512], F32, tag="op", name="warm")
                    nc.tensor.matmul(wps[:], wo_sb[:, 0, 0:128], wo_sb[:, 0, 0:512],
                                     start=True, stop=True)

                def normalize(uts, p, q0):
                    dens, us = [], []
                    for hh in range(2):
                        den1 = npool.tile([1, 512], F32, tag="den", name=f"den{hh}")
                        nc.vector.tensor_scalar_add(den1[:], uts[hh][64:65, :], 1.0)
                        u = npool.tile([64, 512], F32, tag="u", name=f"u{hh}")
                        nc.vector.tensor_copy(u[:], uts[hh][0:64, :])
                        dens.append(den1)
                        us.append(u)
                    for hh in range(2):
                        po = 64 * hh
                        r = npool.tile([1, 512], F32, tag="r")
                        nc.vector.reciprocal_approx_fast(r[:], dens[hh][:])
                        rb = npool.tile([64, 512], F32, tag="rb")
                        nc.gpsimd.partition_broadcast(rb[:], r[:])
                        nc.vector.tensor_mul(
                            at_sb[po:po + 64, p, q0:q0 + 512], us[hh][:], rb[:])

                sched = [(p, qq, i) for p in range(2) for qq in range(4)
                         for i in range(KT16)]
                quarters = {}
                prev = None
                for g in range(len(sched) + 1):
                    if g < len(sched):
                        p, qq, i = sched[g]
                        if i == 0:
                            quarters[(p, qq)] = (
                                put.tile([65, 512], F32, tag="ut", name="ut0"),
                                put.tile([65, 512], F32, tag="ut", name="ut1"),
                            )
                        q0 = qq * 512
                        sc = psc.tile([128, 1024], F32, tag="sc")
                        for hh in range(2):
                            nc.tensor.matmul(
                                sc[:, hh * 512:(hh + 1) * 512],
                                kt_sb[64 * hh:64 * hh + 64, p, i * 128:(i + 1) * 128],
                                qt_sb[64 * hh:64 * hh + 64, p, q0:q0 + 512],
                                start=True, stop=True,
                            )
                        ex = expp.tile([128, 1024], F16, tag="ex")
                        nc.scalar.activation(out=ex[:], in_=sc[:], func=EXP)
                        if g == 0:
                            vproj_pair(0)   # k-tiles 0,1 behind the first exp
                        cur = (p, qq, i, ex)
                    else:
                        cur = None
                    if prev is not None:
                        pp, pqq, pi, pex = prev
                        fq = pp == 0 and pqq == 0
                        if fq and pi % 2 == 1 and pi < KT16 - 2:
                            vproj_pair(pi + 1)  # stays ahead of the V-MMs
                        elif not fq:
                            ot = (pqq - 1) * 4 + (pi - 8) // 2 \
                                if pp == 1 and pqq > 0 and pi >= 8 and pi % 2 == 0 \
                                else None
                            if ot is not None:
                                outproj_t(ot)
                            elif pout is not None and pi % 4 == 0:
                                dummy_mm()
                        uts = quarters[(pp, pqq)]
                        for hh in range(2):
                            h = 2 * pp + hh
                            nc.tensor.matmul(
                                uts[hh][:],
                                v_sb[:, pi, h * VW:(h + 1) * VW],
                                pex[:, hh * 512:(hh + 1) * 512],
                                start=(pi == 0), stop=(pi == KT16 - 1),
                            )
                        if pi == KT16 - 1:
                            if fq:
                                pv_ctx.__exit__(None, None, None)
                                pout_ctx = tc.tile_pool(name="pout", bufs=2,
                                                        space="PSUM")
                                pout = pout_ctx.__enter__()
                            normalize(uts, pp, pqq * 512)
                            del quarters[(pp, pqq)]
                    prev = cur
                # final q-quarter's out-projection (ACT is idle by now)
                for t in range(12, 16):
                    outproj_t(t, act_copy=True)
                pout_ctx.__exit__(None, None, None)

    nc.finalize()
    return nc


def kernel(query, key, value, Wq, bq, Wk, bk, Wv, bv, Wo, bo):
    global LAST_RESULT
    if "nc" not in _CACHE:
        _CACHE["nc"] = _build()
    nc = _CACHE["nc"]

    query = np.asarray(query, np.float32)
    key = np.asarray(key, np.float32)
    value = np.asarray(value, np.float32)
    Wq = np.asarray(Wq, np.float32)
    Wk = np.asarray(Wk, np.float32)
    Wv = np.asarray(Wv, np.float32)
    Wo = np.asarray(Wo, np.float32)
    bq = np.asarray(bq, np.float32)
    bk = np.asarray(bk, np.float32)
    bv = np.asarray(bv, np.float32)
    bo = np.asarray(bo, np.float32)

    xqT = [np.ascontiguousarray(query[b].T).astype(np.float16) for b in range(B)]
    xkT = [np.ascontiguousarray(key[b].T).astype(np.float16) for b in range(B)]
    xvT = [np.ascontiguousarray(value[b].T).astype(np.float16) for b in range(B)]

    ones1 = np.ones((1, 128), np.float16)
    in_maps = []
    for c in range(8):
        b, hg = c // 4, c % 4
        r0 = hg * CD
        wq_s = np.ascontiguousarray((Wq[r0:r0 + CD, :] * SCALE).T).astype(np.float16)
        wk_s = np.ascontiguousarray(Wk[r0:r0 + CD, :].T).astype(np.float16)
        wo_s = np.ascontiguousarray(Wo[:, r0:r0 + CD].T).astype(np.float16)
        bq_s = np.ascontiguousarray((bq[r0:r0 + CD] * SCALE).reshape(2, 128).T)  # [128,2]
        bk_s = np.ascontiguousarray(bk[r0:r0 + CD].reshape(2, 128).T)
        # V weights/bias in 260-layout: [64 cols of head | bias-1 col] x4
        wv260 = np.zeros((DM, W260), np.float32)
        bv260 = np.zeros((1, W260), np.float32)
        for hh in range(HLOC):
            wv260[:, hh * VW:hh * VW + HD] = Wv[r0 + hh * HD:r0 + (hh + 1) * HD, :].T
            bv260[0, hh * VW:hh * VW + HD] = bv[r0 + hh * HD:r0 + (hh + 1) * HD]
            bv260[0, hh * VW + HD] = 1.0
        in_maps.append({
            "xq": xqT[b], "xk": xkT[b], "xv": xvT[b],
            "wq": wq_s, "wk": wk_s, "wv": np.ascontiguousarray(wv260).astype(np.float16),
            "wo": wo_s, "bq": bq_s, "bk": bk_s, "bv": bv260.astype(np.float16),
            "ones1": ones1,
        })

    res = run_bass_kernel_spmd(nc, in_maps, core_ids=list(range(8)))
    LAST_RESULT = res

    out = np.empty((B, S, DM), np.float32)
    for b in range(B):
        acc = np.zeros((S, DM), np.float64)
        for hg in range(4):
            acc += res.results[b * 4 + hg]["out"].astype(np.float64)
        out[b] = (acc + bo.astype(np.float64)).astype(np.float32)
    return out



# revision 9
# speedup vs baseline: 1.4829x; 1.0139x over previous
"""Multi-head attention (softmax+1) for TRN2, 8 NeuronCores.

Sharding: data-parallel over batch B=2 (4 cores per batch) x tensor-parallel
over the 16 heads (4 heads per core).  Each core computes its 4 heads'
QKV projections, attention, and a partial output projection; the host sums
the 4 partials per batch and adds the output bias.

Per-core kernel (S=2048, DM=1024, HD=64, Hloc=4):
  QT[d,q] / KT[d,k] head-transposed layouts from x^T inputs (PE matmuls),
  V'[k, 4*65] natural layout with a ones column per head (denominator trick),
  scores^T[k,q] -> exp on ACT (scale folded into Wq) -> U^T = V'^T @ expT
  (row 64 of each head's block = softmax denominator), normalization via
  1/(1+den) broadcast (GPSIMD partition_broadcast), partial out-projection.

All matmuls run in float16 (1 cycle/row on the PE).  Matmuls are emitted in
concurrent row-group pairs wherever possible (head-pair scores on partitions
0:64/64:128; projections as half-contraction pairs alternating row groups),
which hides LDWEIGHTS and doubles array occupancy.  The attention phase is
ACT(exp)-bound and software-pipelined one chunk ahead (scores/exp lead the
V-accumulation) so the scalar engine never starves across quarter
boundaries; V-projection / out-projection / dummy matmuls fill the PE to
keep the HAM clock-gate at 8/8.
"""

import sys

if "/opt/trn_rl_repo" not in sys.path:
    sys.path.insert(0, "/opt/trn_rl_repo")

import numpy as np

import concourse.bass as bass
import concourse.mybir as mybir
import concourse.tile as tile
from concourse import bacc
from concourse.bass_utils import run_bass_kernel_spmd

F32 = mybir.dt.float32
F16 = mybir.dt.float16
EXP = mybir.ActivationFunctionType.Exp

B, S, DM = 2, 2048, 1024
H, HD = 16, 64
SCALE = HD ** -0.5
HLOC = 4              # heads per core
CD = HLOC * HD        # 256 local head dims
VW = HD + 1           # 65: V columns + ones column per head
MC = DM // 128        # 8 contraction chunks for projections
KT16 = S // 128       # 16 sequence tiles
W260 = HLOC * VW      # 260

_CACHE = {}
LAST_RESULT = None


def _build():
    nc = bacc.Bacc()
    dp = nc.declare_dram_parameter
    xq_d = dp("xq", [DM, S], F16, isOutput=False)    # query[b]^T
    xk_d = dp("xk", [DM, S], F16, isOutput=False)
    xv_d = dp("xv", [DM, S], F16, isOutput=False)
    wq_d = dp("wq", [DM, CD], F16, isOutput=False)   # (SCALE * Wq_shard)^T
    wk_d = dp("wk", [DM, CD], F16, isOutput=False)   # Wk_shard^T
    wv_d = dp("wv", [DM, W260], F16, isOutput=False)  # Wv^T 260-layout, zeros in ones-cols
    wo_d = dp("wo", [CD, DM], F16, isOutput=False)   # Wo_shard^T
    bq_d = dp("bq", [128, 2], F32, isOutput=False)   # bias cols per 128-pair (SCALE-folded)
    bk_d = dp("bk", [128, 2], F32, isOutput=False)
    bv_d = dp("bv", [1, W260], F16, isOutput=False)  # [bv_h | 1.0] blocks
    on_d = dp("ones1", [1, 128], F16, isOutput=False)
    out_d = dp("out", [S, DM], F16, isOutput=True)   # partial (pre-bo) projection

    with tile.TileContext(nc) as tc:
        with tc.tile_pool(name="weights", bufs=1) as wpool, \
             tc.tile_pool(name="persist", bufs=1) as perst:
            wq_sb = wpool.tile([128, MC, CD], F16)
            wk_sb = wpool.tile([128, MC, CD], F16)
            wv_sb = wpool.tile([128, MC, W260], F16)
            wo_sb = wpool.tile([128, 2, DM], F16)
            bq_sb = wpool.tile([128, 2], F32)
            bk_sb = wpool.tile([128, 2], F32)
            bv_sb = wpool.tile([1, W260], F16)
            on_sb = wpool.tile([1, 128], F16)

            qt_sb = perst.tile([128, 2, S], F16)   # [d(2 heads), pair, q]
            kt_sb = perst.tile([128, 2, S], F16)
            v_sb = perst.tile([128, KT16, W260], F16)  # [k, ktile, 4*(V|1)]
            at_sb = perst.tile([128, 2, S], F16)   # normalized attn out^T
            xv_sb = perst.tile([128, MC, S], F16)  # resident value^T chunks

            # ------------- Phase 1: Q and K projections ----------------
            # Half-contraction matmul pairs on alternating row groups: the
            # second matmul's LDWEIGHTS overlaps the first's stream.
            with tc.tile_pool(name="xs", bufs=16) as xs, \
                 tc.tile_pool(name="pproj", bufs=8, space="PSUM") as pproj:
                nc.sync.dma_start(
                    out=wq_sb[:], in_=wq_d.ap().rearrange("(m p) c -> p m c", m=MC))
                nc.sync.dma_start(out=bq_sb[:], in_=bq_d.ap())
                nc.sync.dma_start(
                    out=wk_sb[:], in_=wk_d.ap().rearrange("(m p) c -> p m c", m=MC))
                nc.sync.dma_start(out=bk_sb[:], in_=bk_d.ap())
                for src_d, w_sb, b_sb, dst in (
                    (xq_d, wq_sb, bq_sb, qt_sb),
                    (xk_d, wk_sb, bk_sb, kt_sb),
                ):
                    pss = [pproj.tile([128, 512], F32, tag="ps", name=f"ps{k}")
                           for k in range(8)]
                    xts = []
                    for m in range(MC):
                        xt = xs.tile([128, S], F16, tag="xs", name=f"xt{m}")
                        nc.sync.dma_start(out=xt[:], in_=src_d.ap()[m * 128:(m + 1) * 128, :])
                        xts.append(xt)
                    for m in range(MC):
                        xt = xts[m]
                        st, sp = (m == 0), (m == MC - 1)
                        for p in range(2):
                            for j in range(4):
                                nc.tensor.matmul(
                                    pss[p * 4 + j][:],
                                    w_sb[:, m, p * 128:(p + 1) * 128],
                                    xt[:, j * 512:(j + 1) * 512],
                                    start=st, stop=sp,
                                )
                    for p in range(2):
                        for j in range(4):
                            nc.vector.tensor_scalar_add(
                                dst[:, p, j * 512:(j + 1) * 512],
                                pss[p * 4 + j][:], b_sb[:, p:p + 1],
                            )
                # stage V weights/input + wo for the attention phase
                nc.sync.dma_start(
                    out=wv_sb[:], in_=wv_d.ap().rearrange("(m p) c -> p m c", m=MC))
                nc.sync.dma_start(out=bv_sb[:], in_=bv_d.ap())
                nc.sync.dma_start(out=on_sb[:], in_=on_d.ap())
                nc.sync.dma_start(
                    out=xv_sb[:], in_=xv_d.ap().rearrange("(m p) c -> p m c", m=MC))
                nc.sync.dma_start(
                    out=wo_sb[:], in_=wo_d.ap().rearrange("(k p) c -> p k c", k=2))

            # ------------- Phase 2: attention, software-pipelined -----------
            with tc.tile_pool(name="psc", bufs=2, space="PSUM") as psc, \
                 tc.tile_pool(name="put", bufs=2, space="PSUM") as put, \
                 tc.tile_pool(name="expp", bufs=4) as expp, \
                 tc.tile_pool(name="obuf", bufs=3) as obuf, \
                 tc.tile_pool(name="npool", bufs=3) as npool:

                pout = None
                pv_ctx = tc.tile_pool(name="pv", bufs=2, space="PSUM")
                pv = pv_ctx.__enter__()

                def vproj_one(kt):
                    """V projection for one k-tile."""
                    vps = pv.tile([128, W260], F32, tag="vps", name="vps")
                    nc.tensor.matmul(vps[:], on_sb[:], bv_sb[:], start=True, stop=False)
                    for m in range(MC):
                        nc.tensor.matmul(
                            vps[:],
                            xv_sb[:, m, kt * 128:(kt + 1) * 128],
                            wv_sb[:, m, :],
                            start=False, stop=(m == MC - 1),
                        )
                    nc.vector.tensor_copy(v_sb[:, kt, :], vps[:])

                def outproj_t(t, act_copy=False):
                    ob = obuf.tile([128, DM], F16, tag="ob", name="ob")
                    ops = [pout.tile([128, 512], F32, tag="op", name=f"op{n}")
                           for n in range(2)]
                    for cc in range(2):
                        for n in range(2):
                            nc.tensor.matmul(
                                ops[n][:],
                                at_sb[:, cc, t * 128:(t + 1) * 128],
                                wo_sb[:, cc, n * 512:(n + 1) * 512],
                                start=(cc == 0), stop=(cc == 1),
                            )
                    nc.vector.tensor_copy(ob[:, 0:512], ops[0][:])
                    if act_copy:
                        nc.scalar.copy(ob[:, 512:1024], ops[1][:])
                    else:
                        nc.vector.tensor_copy(ob[:, 512:1024], ops[1][:])
                    nc.sync.dma_start(
                        out=out_d.ap()[t * 128:(t + 1) * 128, :], in_=ob[:],
                    )

                def dummy_mm():
                    wps = pout.tile([128, 512], F32, tag="op", name="warm")
                    nc.tensor.matmul(wps[:], wo_sb[:, 0, 0:128], wo_sb[:, 0, 0:512],
                                     start=True, stop=True)

                def normalize(uts, p, q0):
                    dens, us = [], []
                    for hh in range(2):
                        den1 = npool.tile([1, 512], F32, tag="den", name=f"den{hh}")
                        nc.vector.tensor_scalar_add(den1[:], uts[hh][64:65, :], 1.0)
                        u = npool.tile([64, 512], F32, tag="u", name=f"u{hh}")
                        nc.vector.tensor_copy(u[:], uts[hh][0:64, :])
                        dens.append(den1)
                        us.append(u)
                    for hh in range(2):
                        po = 64 * hh
                        r = npool.tile([1, 512], F32, tag="r")
                        nc.vector.reciprocal_approx_fast(r[:], dens[hh][:])
                        rb = npool.tile([64, 512], F32, tag="rb")
                        nc.gpsimd.partition_broadcast(rb[:], r[:])
                        nc.vector.tensor_mul(
                            at_sb[po:po + 64, p, q0:q0 + 512], us[hh][:], rb[:])

                sched = [(p, qq, i) for p in range(2) for qq in range(4)
                         for i in range(KT16)]
                quarters = {}
                prev = None
                for g in range(len(sched) + 1):
                    if g < len(sched):
                        p, qq, i = sched[g]
                        if i == 0:
                            quarters[(p, qq)] = (
                                put.tile([65, 512], F32, tag="ut", name="ut0"),
                                put.tile([65, 512], F32, tag="ut", name="ut1"),
                            )
                        q0 = qq * 512
                        sc = psc.tile([128, 1024], F32, tag="sc")
                        for hh in range(2):
                            nc.tensor.matmul(
                                sc[:, hh * 512:(hh + 1) * 512],
                                kt_sb[64 * hh:64 * hh + 64, p, i * 128:(i + 1) * 128],
                                qt_sb[64 * hh:64 * hh + 64, p, q0:q0 + 512],
                                start=True, stop=True,
                            )
                        ex = expp.tile([128, 1024], F16, tag="ex")
                        nc.scalar.activation(out=ex[:], in_=sc[:], func=EXP)
                        if g == 0:
                            vproj_one(0)    # k-tiles 0,1 behind the first exp
                            vproj_one(1)
                        cur = (p, qq, i, ex)
                    else:
                        cur = None
                    if prev is not None:
                        pp, pqq, pi, pex = prev
                        fq = pp == 0 and pqq == 0
                        if fq and pi < KT16 - 2:
                            vproj_one(pi + 2)   # stays ahead of the V-MMs
                        elif not fq:
                            ot = (pqq - 1) * 4 + (pi - 8) // 2 \
                                if pp == 1 and pqq > 0 and pi >= 8 and pi % 2 == 0 \
                                else None
                            if ot is not None:
                                outproj_t(ot)
                            elif pout is not None and pi % 4 == 0:
                                dummy_mm()
                        uts = quarters[(pp, pqq)]
                        for hh in range(2):
                            h = 2 * pp + hh
                            nc.tensor.matmul(
                                uts[hh][:],
                                v_sb[:, pi, h * VW:(h + 1) * VW],
                                pex[:, hh * 512:(hh + 1) * 512],
                                start=(pi == 0), stop=(pi == KT16 - 1),
                            )
                        if pi == KT16 - 1:
                            if fq:
                                pv_ctx.__exit__(None, None, None)
                                pout_ctx = tc.tile_pool(name="pout", bufs=2,
                                                        space="PSUM")
                                pout = pout_ctx.__enter__()
                            normalize(uts, pp, pqq * 512)
                            del quarters[(pp, pqq)]
                    prev = cur
                # final q-quarter's out-projection (ACT is idle by now)
                for t in range(12, 16):
                    outproj_t(t, act_copy=True)
                pout_ctx.__exit__(None, None, None)

    nc.finalize()
    return nc


def kernel(query, key, value, Wq, bq, Wk, bk, Wv, bv, Wo, bo):
    global LAST_RESULT
    if "nc" not in _CACHE:
        _CACHE["nc"] = _build()
    nc = _CACHE["nc"]

    query = np.asarray(query, np.float32)
    key = np.asarray(key, np.float32)
    value = np.asarray(value, np.float32)
    Wq = np.asarray(Wq, np.float32)
    Wk = np.asarray(Wk, np.float32)
    Wv = np.asarray(Wv, np.float32)
    Wo = np.asarray(Wo, np.float32)
    bq = np.asarray(bq, np.float32)
    bk = np.asarray(bk, np.float32)
    bv = np.asarray(bv, np.float32)
    bo = np.asarray(bo, np.float32)

    xqT = [np.ascontiguousarray(query[b].T).astype(np.float16) for b in range(B)]
    xkT = [np.ascontiguousarray(key[b].T).astype(np.float16) for b in range(B)]
    xvT = [np.ascontiguousarray(value[b].T).astype(np.float16) for b in range(B)]

    ones1 = np.ones((1, 128), np.float16)
    in_maps = []
    for c in range(8):
        b, hg = c // 4, c % 4
        r0 = hg * CD
        wq_s = np.ascontiguousarray((Wq[r0:r0 + CD, :] * SCALE).T).astype(np.float16)
        wk_s = np.ascontiguousarray(Wk[r0:r0 + CD, :].T).astype(np.float16)
        wo_s = np.ascontiguousarray(Wo[:, r0:r0 + CD].T).astype(np.float16)
        bq_s = np.ascontiguousarray((bq[r0:r0 + CD] * SCALE).reshape(2, 128).T)  # [128,2]
        bk_s = np.ascontiguousarray(bk[r0:r0 + CD].reshape(2, 128).T)
        # V weights/bias in 260-layout: [64 cols of head | bias-1 col] x4
        wv260 = np.zeros((DM, W260), np.float32)
        bv260 = np.zeros((1, W260), np.float32)
        for hh in range(HLOC):
            wv260[:, hh * VW:hh * VW + HD] = Wv[r0 + hh * HD:r0 + (hh + 1) * HD, :].T
            bv260[0, hh * VW:hh * VW + HD] = bv[r0 + hh * HD:r0 + (hh + 1) * HD]
            bv260[0, hh * VW + HD] = 1.0
        in_maps.append({
            "xq": xqT[b], "xk": xkT[b], "xv": xvT[b],
            "wq": wq_s, "wk": wk_s, "wv": np.ascontiguousarray(wv260).astype(np.float16),
            "wo": wo_s, "bq": bq_s, "bk": bk_s, "bv": bv260.astype(np.float16),
            "ones1": ones1,
        })

    res = run_bass_kernel_spmd(nc, in_maps, core_ids=list(range(8)))
    LAST_RESULT = res

    out = np.empty((B, S, DM), np.float32)
    for b in range(B):
        acc = np.zeros((S, DM), np.float64)
        for hg in range(4):
            acc += res.results[b * 4 + hg]["out"].astype(np.float64)
        out[b] = (acc + bo.astype(np.float64)).astype(np.float32)
    return out



# revision 10
# speedup vs baseline: 1.5029x; 1.0135x over previous
"""Multi-head attention (softmax+1) for TRN2, 8 NeuronCores.

Sharding: data-parallel over batch B=2 (4 cores per batch) x tensor-parallel
over the 16 heads (4 heads per core).  Each core computes its 4 heads'
QKV projections, attention, and a partial output projection; the host sums
the 4 partials per batch and adds the output bias.

Per-core kernel (S=2048, DM=1024, HD=64, Hloc=4):
  QT[d,q] / KT[d,k] head-transposed layouts from x^T inputs (PE matmuls),
  V'[k, 4*65] natural layout with a ones column per head (denominator trick),
  scores^T[k,q] -> exp on ACT (scale folded into Wq) -> U^T = V'^T @ expT
  (row 64 of each head's block = softmax denominator), normalization via
  1/(1+den) broadcast (GPSIMD partition_broadcast), partial out-projection.

All matmuls run in float16 (1 cycle/row on the PE).  Matmuls are emitted in
concurrent row-group pairs wherever possible (head-pair scores on partitions
0:64/64:128; projections as half-contraction pairs alternating row groups),
which hides LDWEIGHTS and doubles array occupancy.  The attention phase is
ACT(exp)-bound and software-pipelined one chunk ahead (scores/exp lead the
V-accumulation) so the scalar engine never starves across quarter
boundaries; V-projection / out-projection / dummy matmuls fill the PE to
keep the HAM clock-gate at 8/8.
"""

import sys

if "/opt/trn_rl_repo" not in sys.path:
    sys.path.insert(0, "/opt/trn_rl_repo")

import numpy as np

import concourse.bass as bass
import concourse.mybir as mybir
import concourse.tile as tile
from concourse import bacc
from concourse.bass_utils import run_bass_kernel_spmd

F32 = mybir.dt.float32
F16 = mybir.dt.float16
EXP = mybir.ActivationFunctionType.Exp

B, S, DM = 2, 2048, 1024
H, HD = 16, 64
SCALE = HD ** -0.5
HLOC = 4              # heads per core
CD = HLOC * HD        # 256 local head dims
VW = HD + 1           # 65: V columns + ones column per head
MC = DM // 128        # 8 contraction chunks for projections
KT16 = S // 128       # 16 sequence tiles
W260 = HLOC * VW      # 260

_CACHE = {}
LAST_RESULT = None


def _build():
    nc = bacc.Bacc()
    dp = nc.declare_dram_parameter
    xq_d = dp("xq", [DM, S], F16, isOutput=False)    # query[b]^T
    xk_d = dp("xk", [DM, S], F16, isOutput=False)
    xv_d = dp("xv", [DM, S], F16, isOutput=False)
    wq_d = dp("wq", [DM, CD], F16, isOutput=False)   # (SCALE * Wq_shard)^T
    wk_d = dp("wk", [DM, CD], F16, isOutput=False)   # Wk_shard^T
    wv_d = dp("wv", [DM, W260], F16, isOutput=False)  # Wv^T 260-layout, zeros in ones-cols
    wo_d = dp("wo", [CD, DM], F16, isOutput=False)   # Wo_shard^T
    bq_d = dp("bq", [128, 2], F32, isOutput=False)   # bias cols per 128-pair (SCALE-folded)
    bk_d = dp("bk", [128, 2], F32, isOutput=False)
    bv_d = dp("bv", [1, W260], F16, isOutput=False)  # [bv_h | 1.0] blocks
    on_d = dp("ones1", [1, 128], F16, isOutput=False)
    out_d = dp("out", [S, DM], F16, isOutput=True)   # partial (pre-bo) projection

    with tile.TileContext(nc) as tc:
        with tc.tile_pool(name="weights", bufs=1) as wpool, \
             tc.tile_pool(name="persist", bufs=1) as perst:
            wq_sb = wpool.tile([128, MC, CD], F16)
            wk_sb = wpool.tile([128, MC, CD], F16)
            wv_sb = wpool.tile([128, MC, W260], F16)
            wo_sb = wpool.tile([128, 2, DM], F16)
            bq_sb = wpool.tile([128, 2], F32)
            bk_sb = wpool.tile([128, 2], F32)
            bv_sb = wpool.tile([1, W260], F16)
            on_sb = wpool.tile([1, 128], F16)

            qt_sb = perst.tile([128, 2, S], F16)   # [d(2 heads), pair, q]
            kt_sb = perst.tile([128, 2, S], F16)
            v_sb = perst.tile([128, KT16, W260], F16)  # [k, ktile, 4*(V|1)]
            at_sb = perst.tile([128, 2, S], F16)   # normalized attn out^T
            xv_sb = perst.tile([128, MC, S], F16)  # resident value^T chunks

            # ------------- Phase 1: Q and K projections ----------------
            # Half-contraction matmul pairs on alternating row groups: the
            # second matmul's LDWEIGHTS overlaps the first's stream.
            with tc.tile_pool(name="xs", bufs=16) as xs, \
                 tc.tile_pool(name="pproj", bufs=8, space="PSUM") as pproj:
                nc.sync.dma_start(
                    out=wq_sb[:], in_=wq_d.ap().rearrange("(m p) c -> p m c", m=MC))
                nc.sync.dma_start(out=bq_sb[:], in_=bq_d.ap())
                for src_d, w_sb, b_sb, dst in (
                    (xq_d, wq_sb, bq_sb, qt_sb),
                    (xk_d, wk_sb, bk_sb, kt_sb),
                ):
                    pss = [pproj.tile([128, 512], F32, tag="ps", name=f"ps{k}")
                           for k in range(8)]
                    xts = []
                    for m in range(MC):
                        xt = xs.tile([128, S], F16, tag="xs", name=f"xt{m}")
                        nc.sync.dma_start(out=xt[:], in_=src_d.ap()[m * 128:(m + 1) * 128, :])
                        xts.append(xt)
                    if dst is qt_sb:
                        # K weights enqueue after the xq chunks so xq streams first
                        nc.sync.dma_start(
                            out=wk_sb[:],
                            in_=wk_d.ap().rearrange("(m p) c -> p m c", m=MC))
                        nc.sync.dma_start(out=bk_sb[:], in_=bk_d.ap())
                    for m in range(MC):
                        xt = xts[m]
                        st, sp = (m == 0), (m == MC - 1)
                        for p in range(2):
                            for j in range(4):
                                nc.tensor.matmul(
                                    pss[p * 4 + j][:],
                                    w_sb[:, m, p * 128:(p + 1) * 128],
                                    xt[:, j * 512:(j + 1) * 512],
                                    start=st, stop=sp,
                                )
                    for p in range(2):
                        for j in range(4):
                            nc.vector.tensor_scalar_add(
                                dst[:, p, j * 512:(j + 1) * 512],
                                pss[p * 4 + j][:], b_sb[:, p:p + 1],
                            )
                # stage V weights/input + wo for the attention phase
                nc.sync.dma_start(
                    out=wv_sb[:], in_=wv_d.ap().rearrange("(m p) c -> p m c", m=MC))
                nc.sync.dma_start(out=bv_sb[:], in_=bv_d.ap())
                nc.sync.dma_start(out=on_sb[:], in_=on_d.ap())
                for m in range(MC):
                    nc.sync.dma_start(out=xv_sb[:, m, :],
                                      in_=xv_d.ap()[m * 128:(m + 1) * 128, :])
                nc.sync.dma_start(
                    out=wo_sb[:], in_=wo_d.ap().rearrange("(k p) c -> p k c", k=2))

            # ------------- Phase 2: attention, software-pipelined -----------
            with tc.tile_pool(name="psc", bufs=2, space="PSUM") as psc, \
                 tc.tile_pool(name="put", bufs=2, space="PSUM") as put, \
                 tc.tile_pool(name="expp", bufs=4) as expp, \
                 tc.tile_pool(name="obuf", bufs=3) as obuf, \
                 tc.tile_pool(name="npool", bufs=3) as npool:

                pout = None
                pv_ctx = tc.tile_pool(name="pv", bufs=2, space="PSUM")
                pv = pv_ctx.__enter__()

                def vproj_one(kt):
                    """V projection for one k-tile."""
                    vps = pv.tile([128, W260], F32, tag="vps", name="vps")
                    nc.tensor.matmul(vps[:], on_sb[:], bv_sb[:], start=True, stop=False)
                    for m in range(MC):
                        nc.tensor.matmul(
                            vps[:],
                            xv_sb[:, m, kt * 128:(kt + 1) * 128],
                            wv_sb[:, m, :],
                            start=False, stop=(m == MC - 1),
                        )
                    nc.vector.tensor_copy(v_sb[:, kt, :], vps[:])

                ob_tiles = {}

                def outproj_half(t, n, act_copy=False):
                    ob = ob_tiles.get(t)
                    if ob is None:
                        ob = obuf.tile([128, DM], F16, tag="ob", name="ob")
                        ob_tiles[t] = ob
                    op = pout.tile([128, 512], F32, tag="op", name="op")
                    for cc in range(2):
                        nc.tensor.matmul(
                            op[:],
                            at_sb[:, cc, t * 128:(t + 1) * 128],
                            wo_sb[:, cc, n * 512:(n + 1) * 512],
                            start=(cc == 0), stop=(cc == 1),
                        )
                    if act_copy and n == 1:
                        nc.scalar.copy(ob[:, n * 512:(n + 1) * 512], op[:])
                    else:
                        nc.vector.tensor_copy(ob[:, n * 512:(n + 1) * 512], op[:])
                    if n == 1:
                        nc.sync.dma_start(
                            out=out_d.ap()[t * 128:(t + 1) * 128, :], in_=ob[:])
                        del ob_tiles[t]

                def dummy_mm():
                    wps = pout.tile([128, 512], F32, tag="op", name="warm")
                    nc.tensor.matmul(wps[:], wo_sb[:, 0, 0:128], wo_sb[:, 0, 0:512],
                                     start=True, stop=True)

                def normalize(uts, p, q0):
                    dens, us = [], []
                    for hh in range(2):
                        den1 = npool.tile([1, 512], F32, tag="den", name=f"den{hh}")
                        nc.vector.tensor_scalar_add(den1[:], uts[hh][64:65, :], 1.0)
                        u = npool.tile([64, 512], F32, tag="u", name=f"u{hh}")
                        nc.vector.tensor_copy(u[:], uts[hh][0:64, :])
                        dens.append(den1)
                        us.append(u)
                    for hh in range(2):
                        po = 64 * hh
                        r = npool.tile([1, 512], F32, tag="r")
                        nc.vector.reciprocal_approx_fast(r[:], dens[hh][:])
                        rb = npool.tile([64, 512], F32, tag="rb")
                        nc.gpsimd.partition_broadcast(rb[:], r[:])
                        nc.vector.tensor_mul(
                            at_sb[po:po + 64, p, q0:q0 + 512], us[hh][:], rb[:])

                sched = [(p, qq, i) for p in range(2) for qq in range(4)
                         for i in range(KT16)]
                quarters = {}
                prev = None
                for g in range(len(sched) + 1):
                    if g < len(sched):
                        p, qq, i = sched[g]
                        if i == 0:
                            quarters[(p, qq)] = (
                                put.tile([65, 512], F32, tag="ut", name="ut0"),
                                put.tile([65, 512], F32, tag="ut", name="ut1"),
                            )
                        q0 = qq * 512
                        sc = psc.tile([128, 1024], F32, tag="sc")
                        for hh in range(2):
                            nc.tensor.matmul(
                                sc[:, hh * 512:(hh + 1) * 512],
                                kt_sb[64 * hh:64 * hh + 64, p, i * 128:(i + 1) * 128],
                                qt_sb[64 * hh:64 * hh + 64, p, q0:q0 + 512],
                                start=True, stop=True,
                            )
                        ex = expp.tile([128, 1024], F16, tag="ex")
                        nc.scalar.activation(out=ex[:], in_=sc[:], func=EXP)
                        if g == 0:
                            vproj_one(0)    # k-tiles 0,1 behind the first exp
                            vproj_one(1)
                        cur = (p, qq, i, ex)
                    else:
                        cur = None
                    if prev is not None:
                        pp, pqq, pi, pex = prev
                        fq = pp == 0 and pqq == 0
                        if fq and pi < KT16 - 2:
                            vproj_one(pi + 2)   # stays ahead of the V-MMs
                        elif not fq:
                            if pp == 1 and pqq > 0 and pi >= 8:
                                outproj_half((pqq - 1) * 4 + (pi - 8) // 2,
                                             pi % 2)
                            elif pout is not None and pi % 4 == 0:
                                dummy_mm()
                        uts = quarters[(pp, pqq)]
                        for hh in range(2):
                            h = 2 * pp + hh
                            nc.tensor.matmul(
                                uts[hh][:],
                                v_sb[:, pi, h * VW:(h + 1) * VW],
                                pex[:, hh * 512:(hh + 1) * 512],
                                start=(pi == 0), stop=(pi == KT16 - 1),
                            )
                        if pi == KT16 - 1:
                            if fq:
                                pv_ctx.__exit__(None, None, None)
                                pout_ctx = tc.tile_pool(name="pout", bufs=2,
                                                        space="PSUM")
                                pout = pout_ctx.__enter__()
                            normalize(uts, pp, pqq * 512)
                            del quarters[(pp, pqq)]
                    prev = cur
                # final q-quarter's out-projection (ACT is idle by now)
                for t in range(12, 16):
                    outproj_half(t, 0, act_copy=True)
                    outproj_half(t, 1, act_copy=True)
                pout_ctx.__exit__(None, None, None)

    nc.finalize()
    return nc


def kernel(query, key, value, Wq, bq, Wk, bk, Wv, bv, Wo, bo):
    global LAST_RESULT
    if "nc" not in _CACHE:
        _CACHE["nc"] = _build()
    nc = _CACHE["nc"]

    query = np.asarray(query, np.float32)
    key = np.asarray(key, np.float32)
    value = np.asarray(value, np.float32)
    Wq = np.asarray(Wq, np.float32)
    Wk = np.asarray(Wk, np.float32)
    Wv = np.asarray(Wv, np.float32)
    Wo = np.asarray(Wo, np.float32)
    bq = np.asarray(bq, np.float32)
    bk = np.asarray(bk, np.float32)
    bv = np.asarray(bv, np.float32)
    bo = np.asarray(bo, np.float32)

    xqT = [np.ascontiguousarray(query[b].T).astype(np.float16) for b in range(B)]
    xkT = [np.ascontiguousarray(key[b].T).astype(np.float16) for b in range(B)]
    xvT = [np.ascontiguousarray(value[b].T).astype(np.float16) for b in range(B)]

    ones1 = np.ones((1, 128), np.float16)
    in_maps = []
    for c in range(8):
        b, hg = c // 4, c % 4
        r0 = hg * CD
        wq_s = np.ascontiguousarray((Wq[r0:r0 + CD, :] * SCALE).T).astype(np.float16)
        wk_s = np.ascontiguousarray(Wk[r0:r0 + CD, :].T).astype(np.float16)
        wo_s = np.ascontiguousarray(Wo[:, r0:r0 + CD].T).astype(np.float16)
        bq_s = np.ascontiguousarray((bq[r0:r0 + CD] * SCALE).reshape(2, 128).T)  # [128,2]
        bk_s = np.ascontiguousarray(bk[r0:r0 + CD].reshape(2, 128).T)
        # V weights/bias in 260-layout: [64 cols of head | bias-1 col] x4
        wv260 = np.zeros((DM, W260), np.float32)
        bv260 = np.zeros((1, W260), np.float32)
        for hh in range(HLOC):
            wv260[:, hh * VW:hh * VW + HD] = Wv[r0 + hh * HD:r0 + (hh + 1) * HD, :].T
            bv260[0, hh * VW:hh * VW + HD] = bv[r0 + hh * HD:r0 + (hh + 1) * HD]
            bv260[0, hh * VW + HD] = 1.0
        in_maps.append({
            "xq": xqT[b], "xk": xkT[b], "xv": xvT[b],
            "wq": wq_s, "wk": wk_s, "wv": np.ascontiguousarray(wv260).astype(np.float16),
            "wo": wo_s, "bq": bq_s, "bk": bk_s, "bv": bv260.astype(np.float16),
            "ones1": ones1,
        })

    res = run_bass_kernel_spmd(nc, in_maps, core_ids=list(range(8)))
    LAST_RESULT = res

    out = np.empty((B, S, DM), np.float32)
    for b in range(B):
        acc = np.zeros((S, DM), np.float64)
        for hg in range(4):
            acc += res.results[b * 4 + hg]["out"].astype(np.float64)
        out[b] = (acc + bo.astype(np.float64)).astype(np.float32)
    return out



# revision 11
# speedup vs baseline: 1.5265x; 1.0157x over previous
"""Multi-head attention (softmax+1) for TRN2, 8 NeuronCores.

Sharding: data-parallel over batch B=2 (4 cores per batch) x tensor-parallel
over the 16 heads (4 heads per core).  Each core computes its 4 heads'
QKV projections, attention, and a partial output projection; the host sums
the 4 partials per batch and adds the output bias.

Per-core kernel (S=2048, DM=1024, HD=64, Hloc=4):
  QT[d,q] / KT[d,k] head-transposed layouts from x^T inputs (PE matmuls),
  V'[k, 4*65] natural layout with a ones column per head (denominator trick),
  scores^T[k,q] -> exp on ACT (scale folded into Wq) -> U^T = V'^T @ expT
  (row 64 of each head's block = softmax denominator), normalization via
  1/(1+den) broadcast (GPSIMD partition_broadcast), partial out-projection.

All matmuls run in float16 (1 cycle/row on the PE).  Matmuls are emitted in
concurrent row-group pairs wherever possible (head-pair scores on partitions
0:64/64:128; projections as half-contraction pairs alternating row groups),
which hides LDWEIGHTS and doubles array occupancy.  The attention phase is
ACT(exp)-bound and software-pipelined one chunk ahead (scores/exp lead the
V-accumulation) so the scalar engine never starves across quarter
boundaries; V-projection / out-projection / dummy matmuls fill the PE to
keep the HAM clock-gate at 8/8.
"""

import sys

if "/opt/trn_rl_repo" not in sys.path:
    sys.path.insert(0, "/opt/trn_rl_repo")

import numpy as np

import concourse.bass as bass
import concourse.mybir as mybir
import concourse.tile as tile
from concourse import bacc
from concourse.bass_utils import run_bass_kernel_spmd

F32 = mybir.dt.float32
F16 = mybir.dt.float16
EXP = mybir.ActivationFunctionType.Exp

B, S, DM = 2, 2048, 1024
H, HD = 16, 64
SCALE = HD ** -0.5
HLOC = 4              # heads per core
CD = HLOC * HD        # 256 local head dims
VW = HD + 1           # 65: V columns + ones column per head
MC = DM // 128        # 8 contraction chunks for projections
KT16 = S // 128       # 16 sequence tiles
W260 = HLOC * VW      # 260

_CACHE = {}
LAST_RESULT = None


def _build():
    nc = bacc.Bacc()
    dp = nc.declare_dram_parameter
    xq_d = dp("xq", [DM, S], F16, isOutput=False)    # query[b]^T
    xk_d = dp("xk", [DM, S], F16, isOutput=False)
    xv_d = dp("xv", [DM, S], F16, isOutput=False)
    wq_d = dp("wq", [DM, CD], F16, isOutput=False)   # (SCALE * Wq_shard)^T
    wk_d = dp("wk", [DM, CD], F16, isOutput=False)   # Wk_shard^T
    wv_d = dp("wv", [DM, W260], F16, isOutput=False)  # Wv^T 260-layout, zeros in ones-cols
    wo_d = dp("wo", [CD, DM], F16, isOutput=False)   # Wo_shard^T
    bq_d = dp("bq", [128, 2], F32, isOutput=False)   # bias cols per 128-pair (SCALE-folded)
    bk_d = dp("bk", [128, 2], F32, isOutput=False)
    bv_d = dp("bv", [1, W260], F16, isOutput=False)  # [bv_h | 1.0] blocks
    on_d = dp("ones1", [1, 128], F16, isOutput=False)
    out_d = dp("out", [S, DM], F16, isOutput=True)   # partial (pre-bo) projection

    with tile.TileContext(nc) as tc:
        with tc.tile_pool(name="weights", bufs=1) as wpool, \
             tc.tile_pool(name="persist", bufs=1) as perst:
            wq_sb = wpool.tile([128, MC, CD], F16)
            wk_sb = wpool.tile([128, MC, CD], F16)
            wv_sb = wpool.tile([128, MC, W260], F16)
            wo_sb = wpool.tile([128, 2, DM], F16)
            bq_sb = wpool.tile([128, 2], F32)
            bk_sb = wpool.tile([128, 2], F32)
            bv_sb = wpool.tile([1, W260], F16)
            on_sb = wpool.tile([1, 128], F16)

            qt_sb = perst.tile([128, 2, S], F16)   # [d(2 heads), pair, q]
            kt_sb = perst.tile([128, 2, S], F16)
            v_sb = perst.tile([128, KT16, W260], F16)  # [k, ktile, 4*(V|1)]
            at_sb = perst.tile([128, 2, S], F16)   # normalized attn out^T
            xv_sb = perst.tile([128, MC, S], F16)  # resident value^T chunks

            # ------------- Phase 1: Q and K projections ----------------
            # Half-contraction matmul pairs on alternating row groups: the
            # second matmul's LDWEIGHTS overlaps the first's stream.
            with tc.tile_pool(name="xs", bufs=16) as xs, \
                 tc.tile_pool(name="pproj", bufs=8, space="PSUM") as pproj:
                nc.sync.dma_start(
                    out=wq_sb[:], in_=wq_d.ap().rearrange("(m p) c -> p m c", m=MC))
                nc.sync.dma_start(out=bq_sb[:], in_=bq_d.ap())
                for src_d, w_sb, b_sb, dst in (
                    (xq_d, wq_sb, bq_sb, qt_sb),
                    (xk_d, wk_sb, bk_sb, kt_sb),
                ):
                    pss = [pproj.tile([128, 512], F32, tag="ps", name=f"ps{k}")
                           for k in range(8)]
                    xts = []
                    for m in range(MC):
                        xt = xs.tile([128, S], F16, tag="xs", name=f"xt{m}")
                        nc.sync.dma_start(out=xt[:], in_=src_d.ap()[m * 128:(m + 1) * 128, :])
                        xts.append(xt)
                    if dst is qt_sb:
                        # K weights enqueue after the xq chunks so xq streams first
                        nc.sync.dma_start(
                            out=wk_sb[:],
                            in_=wk_d.ap().rearrange("(m p) c -> p m c", m=MC))
                        nc.sync.dma_start(out=bk_sb[:], in_=bk_d.ap())
                    for m in range(MC):
                        xt = xts[m]
                        st, sp = (m == 0), (m == MC - 1)
                        for p in range(2):
                            for j in range(4):
                                nc.tensor.matmul(
                                    pss[p * 4 + j][:],
                                    w_sb[:, m, p * 128:(p + 1) * 128],
                                    xt[:, j * 512:(j + 1) * 512],
                                    start=st, stop=sp,
                                )
                    for p in range(2):
                        for j in range(4):
                            nc.vector.tensor_scalar_add(
                                dst[:, p, j * 512:(j + 1) * 512],
                                pss[p * 4 + j][:], b_sb[:, p:p + 1],
                            )
                # stage V weights/input + wo for the attention phase
                nc.sync.dma_start(
                    out=wv_sb[:], in_=wv_d.ap().rearrange("(m p) c -> p m c", m=MC))
                nc.sync.dma_start(out=bv_sb[:], in_=bv_d.ap())
                nc.sync.dma_start(out=on_sb[:], in_=on_d.ap())
                for m in range(MC):
                    nc.sync.dma_start(out=xv_sb[:, m, :],
                                      in_=xv_d.ap()[m * 128:(m + 1) * 128, :])
                nc.sync.dma_start(
                    out=wo_sb[:], in_=wo_d.ap().rearrange("(k p) c -> p k c", k=2))

            # ------------- Phase 2: attention, software-pipelined -----------
            with tc.tile_pool(name="psc", bufs=2, space="PSUM") as psc, \
                 tc.tile_pool(name="put", bufs=2, space="PSUM") as put, \
                 tc.tile_pool(name="expp", bufs=4) as expp, \
                 tc.tile_pool(name="obuf", bufs=3) as obuf, \
                 tc.tile_pool(name="npool", bufs=3) as npool:

                pout = None
                pv_ctx = tc.tile_pool(name="pv", bufs=2, space="PSUM")
                pv = pv_ctx.__enter__()

                def vproj_one(kt):
                    """V projection for one k-tile."""
                    vps = pv.tile([128, W260], F32, tag="vps", name="vps")
                    nc.tensor.matmul(vps[:], on_sb[:], bv_sb[:], start=True, stop=False)
                    for m in range(MC):
                        nc.tensor.matmul(
                            vps[:],
                            xv_sb[:, m, kt * 128:(kt + 1) * 128],
                            wv_sb[:, m, :],
                            start=False, stop=(m == MC - 1),
                        )
                    nc.vector.tensor_copy(v_sb[:, kt, :], vps[:])

                ob_tiles = {}

                def outproj_half(t, n, act_copy=False):
                    ob = ob_tiles.get(t)
                    if ob is None:
                        ob = obuf.tile([128, DM], F16, tag="ob", name="ob")
                        ob_tiles[t] = ob
                    op = pout.tile([128, 512], F32, tag="op", name="op")
                    for cc in range(2):
                        nc.tensor.matmul(
                            op[:],
                            at_sb[:, cc, t * 128:(t + 1) * 128],
                            wo_sb[:, cc, n * 512:(n + 1) * 512],
                            start=(cc == 0), stop=(cc == 1),
                        )
                    if act_copy and n == 1:
                        nc.scalar.copy(ob[:, n * 512:(n + 1) * 512], op[:])
                    else:
                        nc.vector.tensor_copy(ob[:, n * 512:(n + 1) * 512], op[:])
                    if n == 1:
                        nc.sync.dma_start(
                            out=out_d.ap()[t * 128:(t + 1) * 128, :], in_=ob[:])
                        del ob_tiles[t]

                def dummy_mm():
                    wps = pout.tile([128, 512], F32, tag="op", name="warm")
                    nc.tensor.matmul(wps[:], wo_sb[:, 0, 0:128], wo_sb[:, 0, 0:512],
                                     start=True, stop=True)

                def normalize(uts, p, q0):
                    dens, us = [], []
                    for hh in range(2):
                        den1 = npool.tile([1, 512], F32, tag="den", name=f"den{hh}")
                        nc.vector.tensor_scalar_add(den1[:], uts[hh][64:65, :], 1.0)
                        u = npool.tile([64, 512], F32, tag="u", name=f"u{hh}")
                        nc.vector.tensor_copy(u[:], uts[hh][0:64, :])
                        dens.append(den1)
                        us.append(u)
                    for hh in range(2):
                        po = 64 * hh
                        r = npool.tile([1, 512], F32, tag="r")
                        nc.vector.reciprocal_approx_fast(r[:], dens[hh][:])
                        rb = npool.tile([64, 512], F32, tag="rb")
                        nc.gpsimd.partition_broadcast(rb[:], r[:])
                        nc.vector.tensor_mul(
                            at_sb[po:po + 64, p, q0:q0 + 512], us[hh][:], rb[:])

                sched = [(p, qq, i) for p in range(2) for qq in range(4)
                         for i in range(KT16)]
                quarters = {}
                hist = []   # per group: [p, qq, i, sc, ex]
                # scores run one group ahead of exp, and the V-accumulation
                # two behind, so the next ACT's input is always ready the
                # moment the previous ACT retires.
                for it in range(len(sched) + 2):
                    if it < len(sched):
                        p, qq, i = sched[it]
                        q0 = qq * 512
                        sc = psc.tile([128, 1024], F32, tag="sc")
                        for hh in range(2):
                            nc.tensor.matmul(
                                sc[:, hh * 512:(hh + 1) * 512],
                                kt_sb[64 * hh:64 * hh + 64, p, i * 128:(i + 1) * 128],
                                qt_sb[64 * hh:64 * hh + 64, p, q0:q0 + 512],
                                start=True, stop=True,
                            )
                        hist.append([p, qq, i, sc, None])
                    if 1 <= it <= len(sched):
                        e = hist[it - 1]
                        ex = expp.tile([128, 1024], F16, tag="ex")
                        nc.scalar.activation(out=ex[:], in_=e[3][:], func=EXP)
                        e[4] = ex
                        if it == 1:
                            vproj_one(0)   # k-tiles 0,1 behind the first exp
                            vproj_one(1)
                    if it >= 2:
                        pp, pqq, pi, _, pex = hist[it - 2]
                        fq = pp == 0 and pqq == 0
                        if fq and pi < KT16 - 2:
                            vproj_one(pi + 2)   # stays ahead of the V-MMs
                        elif not fq:
                            if pp == 1 and pqq > 0 and pi >= 8:
                                outproj_half((pqq - 1) * 4 + (pi - 8) // 2,
                                             pi % 2)
                            elif pout is not None and pi % 4 == 0:
                                dummy_mm()
                        if pi == 0:
                            quarters[(pp, pqq)] = (
                                put.tile([65, 512], F32, tag="ut", name="ut0"),
                                put.tile([65, 512], F32, tag="ut", name="ut1"),
                            )
                        uts = quarters[(pp, pqq)]
                        for hh in range(2):
                            h = 2 * pp + hh
                            nc.tensor.matmul(
                                uts[hh][:],
                                v_sb[:, pi, h * VW:(h + 1) * VW],
                                pex[:, hh * 512:(hh + 1) * 512],
                                start=(pi == 0), stop=(pi == KT16 - 1),
                            )
                        hist[it - 2][4] = None
                        if pi == KT16 - 1:
                            if fq:
                                pv_ctx.__exit__(None, None, None)
                                pout_ctx = tc.tile_pool(name="pout", bufs=2,
                                                        space="PSUM")
                                pout = pout_ctx.__enter__()
                            normalize(uts, pp, pqq * 512)
                            del quarters[(pp, pqq)]
                # final q-quarter's out-projection (ACT is idle by now)
                for t in range(12, 16):
                    outproj_half(t, 0, act_copy=True)
                    outproj_half(t, 1, act_copy=True)
                pout_ctx.__exit__(None, None, None)

    nc.finalize()
    return nc


def kernel(query, key, value, Wq, bq, Wk, bk, Wv, bv, Wo, bo):
    global LAST_RESULT
    if "nc" not in _CACHE:
        _CACHE["nc"] = _build()
    nc = _CACHE["nc"]

    query = np.asarray(query, np.float32)
    key = np.asarray(key, np.float32)
    value = np.asarray(value, np.float32)
    Wq = np.asarray(Wq, np.float32)
    Wk = np.asarray(Wk, np.float32)
    Wv = np.asarray(Wv, np.float32)
    Wo = np.asarray(Wo, np.float32)
    bq = np.asarray(bq, np.float32)
    bk = np.asarray(bk, np.float32)
    bv = np.asarray(bv, np.float32)
    bo = np.asarray(bo, np.float32)

    xqT = [np.ascontiguousarray(query[b].T).astype(np.float16) for b in range(B)]
    xkT = [np.ascontiguousarray(key[b].T).astype(np.float16) for b in range(B)]
    xvT = [np.ascontiguousarray(value[b].T).astype(np.float16) for b in range(B)]

    ones1 = np.ones((1, 128), np.float16)
    in_maps = []
    for c in range(8):
        b, hg = c // 4, c % 4
        r0 = hg * CD
        wq_s = np.ascontiguousarray((Wq[r0:r0 + CD, :] * SCALE).T).astype(np.float16)
        wk_s = np.ascontiguousarray(Wk[r0:r0 + CD, :].T).astype(np.float16)
        wo_s = np.ascontiguousarray(Wo[:, r0:r0 + CD].T).astype(np.float16)
        bq_s = np.ascontiguousarray((bq[r0:r0 + CD] * SCALE).reshape(2, 128).T)  # [128,2]
        bk_s = np.ascontiguousarray(bk[r0:r0 + CD].reshape(2, 128).T)
        # V weights/bias in 260-layout: [64 cols of head | bias-1 col] x4
        wv260 = np.zeros((DM, W260), np.float32)
        bv260 = np.zeros((1, W260), np.float32)
        for hh in range(HLOC):
            wv260[:, hh * VW:hh * VW + HD] = Wv[r0 + hh * HD:r0 + (hh + 1) * HD, :].T
            bv260[0, hh * VW:hh * VW + HD] = bv[r0 + hh * HD:r0 + (hh + 1) * HD]
            bv260[0, hh * VW + HD] = 1.0
        in_maps.append({
            "xq": xqT[b], "xk": xkT[b], "xv": xvT[b],
            "wq": wq_s, "wk": wk_s, "wv": np.ascontiguousarray(wv260).astype(np.float16),
            "wo": wo_s, "bq": bq_s, "bk": bk_s, "bv": bv260.astype(np.float16),
            "ones1": ones1,
        })

    res = run_bass_kernel_spmd(nc, in_maps, core_ids=list(range(8)))
    LAST_RESULT = res

    out = np.empty((B, S, DM), np.float32)
    for b in range(B):
        acc = np.zeros((S, DM), np.float64)
        for hg in range(4):
            acc += res.results[b * 4 + hg]["out"].astype(np.float64)
        out[b] = (acc + bo.astype(np.float64)).astype(np.float32)
    return out

